# revision 15
# baseline (speedup 1.0000x reference)
"""Bidirectional LSTM (B=32, T=512, D=H=512) on 8 Trainium2 NeuronCores.

Strategy:
  - One SPMD program on all 8 cores. Core 0 runs the forward direction,
    core 1 runs the backward direction (same instruction stream, fed
    time-reversed x and the bw weights). Other cores run the same program
    on replicated data; their outputs are ignored.
  - Per step, z is computed into ONE [128, 512] PSUM bank: partition
    32q+b holds the gates for batch row b, hidden chunk q (columns
    [f_q | i_q | o_q | g_q], 4x128). Each contraction matmul (k, q) uses
    a [128, 128] stationary whose only non-zero columns are block q
    (a sliding 4-slot window over a zero-padded arena holding hT_k), so
    all 20 matmuls are standard full-width M=128 accumulations into the
    same bank. The batch-stacked layout lets the elementwise tail run on
    all 128 partitions (3 ACT + 3 DVE ops) instead of 32.
  - h [128,128] (batch+chunk stacked) is PE-transposed per 32-row block
    into hT [128, 4, 32] and copied into the arena's center slots.
  - xp = x @ Wx + b is precomputed: the first T-chunk as a prologue,
    later chunks interleaved into the recurrence steps so the PE stays
    busy during each step's ACT/DVE tail.
  - Output is written as [T, H, B] per direction and reassembled on host.
"""

import os
import sys
import numpy as np

for _p in ("/opt/trn_rl_repo", "/root/.axon_site/_ro/trn_rl_repo"):
    if os.path.isdir(_p) and _p not in sys.path:
        sys.path.insert(0, _p)

B, T, D, H = 32, 512, 512, 512
G = 4 * H
N_CORES = 8

_PROG_CACHE = {}


def _build_program(t_steps=T, reps=1):
    from contextlib import ExitStack
    import concourse.bacc as bacc
    import concourse.tile as tile
    import concourse.mybir as mybir
    from concourse import masks

    f32 = mybir.dt.float32
    f32r = mybir.dt.float32r
    AF = mybir.ActivationFunctionType

    nc = bacc.Bacc("TRN2", target_bir_lowering=False, debug=False,
                   num_devices=N_CORES)

    x_t = nc.dram_tensor("x", [B, t_steps, D], f32, kind="ExternalInput")
    Wx_t = nc.dram_tensor("Wx", [D, G], f32r, kind="ExternalInput")
    Wh_t = nc.dram_tensor("Wh", [H, G], f32r, kind="ExternalInput")
    bv_t = nc.dram_tensor("bv", [G], f32r, kind="ExternalInput")
    idb_t = nc.dram_tensor("idb", [33, 4, 128], f32r, kind="ExternalInput")
    out_t = nc.dram_tensor("out_h", [t_steps, H, B], f32r, kind="ExternalOutput")

    TCH = min(128, t_steps)
    TC = t_steps // TCH
    HH = H // 4  # 128: hidden chunk size

    with tile.TileContext(nc) as tc, ExitStack() as ctx:
        wpool = ctx.enter_context(tc.tile_pool(name="w", bufs=1))
        tpool = ctx.enter_context(tc.tile_pool(name="tmp", bufs=3))
        xpool = ctx.enter_context(tc.tile_pool(name="xin", bufs=2))
        ppool = ctx.enter_context(tc.tile_pool(name="ps", bufs=3, space="PSUM"))
        tppool = ctx.enter_context(tc.tile_pool(name="tps", bufs=2, space="PSUM"))
        p1pool = ctx.enter_context(tc.tile_pool(name="p1s", bufs=1, space="PSUM"))
        dpool = ctx.enter_context(tc.tile_pool(name="dram", bufs=1, space="DRAM"))

        for _rep in range(reps):
            ident = wpool.tile([128, 128], f32)
            masks.make_identity(nc, ident[:])

            idb_sb = wpool.tile([33, 4, 128], f32r)
            nc.sync.dma_start(idb_sb[:], idb_t.ap())

            Wx_sb = wpool.tile([128, 4, G], f32r, tag="Wbig")
            for k in range(4):
                nc.sync.dma_start(Wx_sb[:, k, :], Wx_t.ap()[k * 128:(k + 1) * 128, :])
            Wh_sb = wpool.tile([128, 4, G], f32r, tag="Wbig2")
            for k in range(4):
                nc.sync.dma_start(Wh_sb[:, k, :], Wh_t.ap()[k * 128:(k + 1) * 128, :])

            xp_dram = dpool.tile([t_steps, B, G], f32r)

            # ---- phase-1 quarter-m-tile quantum emitter -------------------
            p1_state = {}

            def emit_p1_quarter(tcki, b, n):
                tsl = slice(tcki * TCH, (tcki + 1) * TCH)
                if n == 0:
                    xt = xpool.tile([TCH, D], f32, tag="xt")
                    nc.sync.dma_start(xt[:], x_t.ap()[b, tsl, :])
                    xT_ps = p1pool.tile([128, 4, TCH], f32, tag="p1")
                    for k in range(4):
                        nc.tensor.transpose(xT_ps[:, k, :],
                                            xt[:, k * 128:(k + 1) * 128],
                                            ident[0:TCH, 0:TCH])
                    xT_sb = xpool.tile([128, 4, TCH], f32r, tag="xT")
                    nc.vector.tensor_copy(xT_sb[:], xT_ps[:])
                    zx = xpool.tile([TCH, G], f32r, tag="zx")
                    p1_state["xT"] = xT_sb
                    p1_state["zx"] = zx
                xT_sb = p1_state["xT"]
                zx = p1_state["zx"]
                zq = p1pool.tile([TCH, 512], f32, tag="p1")
                for k in range(4):
                    nc.tensor.matmul(zq[:], xT_sb[:, k, :],
                                     Wx_sb[:, k, n * 512:(n + 1) * 512],
                                     start=(k == 0), stop=(k == 3))
                nc.vector.tensor_copy(zx[:, n * 512:(n + 1) * 512], zq[:])
                if n == 3:
                    nc.sync.dma_start(xp_dram[tsl, b, :], zx[:])

            # quarter schedule: chunk 0 in the prologue; chunk c>0 spread
            # ~1 quarter/step over steps [TCH*(c-1), TCH*c - 6) so the PE
            # stays busy (HAM-warm) during every step's ACT/DVE tail.
            step_quanta = {}
            span = TCH - 6
            for c in range(1, TC):
                for j in range(4 * B):
                    st = TCH * (c - 1) + (j * span) // (4 * B)
                    step_quanta.setdefault(st, []).append((c, j // 4, j % 4))

            for b in range(B):
                for n in range(4):
                    emit_p1_quarter(0, b, n)

            # ---------------- recurrence ------------------------------------
            # z layout: [128, 512] — partition 32q+b, free [f|i|o|g] x 128
            RING = 6
            PREF = 3  # xr prefetch distance in steps
            xr = wpool.tile([33, RING, G], f32r, tag="xr")
            for s in range(RING):
                nc.sync.dma_start(xr[32:33, s, :], bv_t.ap()[None, :])

            # zero-padded stationary arena: per k, slots [0..6], hT_k at
            # slot 3; stationary for (k, q) = arena[:, k, 3-q:7-q, :]
            arena = wpool.tile([128, 4, 7, B], f32r, tag="arena")
            nc.vector.memset(arena[:].bitcast(f32), 0.0)
            # persistent [c | tanh_g] tile; c lives in [:, 0:HH]
            tgc = wpool.tile([128, 2 * HH], f32, tag="tgc")
            nc.vector.memset(tgc[:], 0.0)

            def emit_idb(t, zp):
                s = t % RING
                for q in range(4):
                    nc.tensor.matmul(zp[:], idb_sb[:, q, :],
                                     xr[:, s, 512 * q:512 * (q + 1)],
                                     start=(q == 0), stop=False)

            # prefetch first xr slots
            for t0 in range(min(PREF, t_steps)):
                nc.sync.dma_start(xr[0:32, t0 % RING, :], xp_dram[t0, :, :])

            zp = ppool.tile([128, 512], f32, tag="z")
            emit_idb(0, zp)

            for t in range(t_steps):
                tp = t + PREF
                if tp < t_steps:
                    nc.sync.dma_start(xr[0:32, tp % RING, :], xp_dram[tp, :, :])

                # contraction: for each k, 4 block-MMs (zero-padded cols)
                for k in range(4):
                    for q in range(4):
                        nc.tensor.matmul(zp[:],
                                         arena[:, k, 3 - q:7 - q, :],
                                         Wh_sb[:, k, 512 * q:512 * (q + 1)],
                                         start=False,
                                         stop=(k == 3 and q == 3))

                # PE work for the tail window, emitted BEFORE the tail so
                # its semaphore waits don't chain on this step's ACT ops:
                # idb round for t+1, p1 quanta, and filler matmuls to keep
                # the HAM clock gate open on late steps.
                if t + 1 < t_steps:
                    zp_next = ppool.tile([128, 512], f32, tag="z")
                    emit_idb(t + 1, zp_next)

                quanta = step_quanta.get(t, ())
                if not quanta and t + 1 < t_steps:
                    warm = p1pool.tile([TCH, 512], f32, tag="p1")
                    for k in range(2):
                        nc.tensor.matmul(warm[:], Wx_sb[:, 0, 0:TCH],
                                         Wx_sb[:, 2, 0:512],
                                         start=(k == 0), stop=(k == 1))

                # --- tail (all ops on 128 partitions) ---
                sfi = tpool.tile([128, 2 * HH], f32, tag="sfi")
                nc.scalar.activation(sfi[:], zp[:, 0:2 * HH], AF.Sigmoid)
                nc.scalar.activation(tgc[:, HH:2 * HH], zp[:, 3 * HH:4 * HH],
                                     AF.Tanh)
                so = tpool.tile([128, HH], f32, tag="so")
                nc.scalar.activation(so[:], zp[:, 2 * HH:3 * HH], AF.Sigmoid)

                t12 = tpool.tile([128, 2 * HH], f32, tag="t12")
                nc.vector.tensor_mul(t12[:], sfi[:], tgc[:])
                nc.vector.tensor_add(tgc[:, 0:HH], t12[:, 0:HH], t12[:, HH:2 * HH])
                tcl = tpool.tile([128, HH], f32, tag="tc")
                nc.scalar.activation(tcl[:], tgc[:, 0:HH], AF.Tanh)
                hs = tpool.tile([128, HH], f32, tag="hs")
                nc.vector.tensor_mul(hs[:], so[:], tcl[:])

                # transpose h into stationary layout, write arena center slots
                hT_ps = tppool.tile([128, 128], f32, tag="tp")
                nc.tensor.transpose(hT_ps[:], hs[:], ident[:])
                nc.vector.tensor_copy(
                    arena[:, :, 3, :],
                    hT_ps[:].rearrange("p (k b) -> p k b", k=4))

                nc.sync.dma_start(out_t.ap()[t].rearrange("(k p) b -> p k b", p=128),
                                  arena[:, :, 3, :])

                for (c, bq, nq) in quanta:
                    emit_p1_quarter(c, bq, nq)

                if t + 1 < t_steps:
                    zp = zp_next

    nc.compile()
    return nc


def _get_program(t_steps=T, reps=1):
    key = (t_steps, reps)
    if key not in _PROG_CACHE:
        _PROG_CACHE[key] = _build_program(t_steps, reps)
    return _PROG_CACHE[key]


def _permute_gates(W, b):
    # reference gate order [i, f, o, g] (each H wide) -> kernel column order
    # per hidden chunk q (128 wide): [f_q | i_q | o_q | g_q]
    i_, f_, o_, g_ = (W[:, k * H:(k + 1) * H] for k in range(4))
    ib, fb, ob, gb = (b[k * H:(k + 1) * H] for k in range(4))
    HH = H // 4
    wcols, bcols = [], []
    for q in range(4):
        sl = slice(q * HH, (q + 1) * HH)
        wcols += [f_[:, sl], i_[:, sl], o_[:, sl], g_[:, sl]]
        bcols += [fb[sl], ib[sl], ob[sl], gb[sl]]
    return (np.ascontiguousarray(np.concatenate(wcols, axis=1)),
            np.ascontiguousarray(np.concatenate(bcols)))


LAST_EXEC_NS = None
LAST_TRACE = None


def _ensure_trace_hook():
    # The container's antenv stub lacks axon_hooks, so trn_boot's NTFF
    # profile hook registration silently degraded. Recreate it here.
    import types
    if "antenv.axon_hooks" in sys.modules:
        return
    mod = types.ModuleType("antenv.axon_hooks")
    _h = [None]
    mod.set_axon_ntff_profile_hook = lambda h: _h.__setitem__(0, h)
    mod.get_axon_ntff_profile_hook = lambda: _h[0]
    sys.modules["antenv.axon_hooks"] = mod
    import antenv
    antenv.axon_hooks = mod
    try:
        from trn_agent_boot.trn_boot import _ntff_profile_via_ctypes
        mod.set_axon_ntff_profile_hook(
            _ntff_profile_via_ctypes("/opt/axon/libaxon_pjrt.so"))
    except Exception as e:
        print(f"trace hook setup failed: {e}", file=sys.stderr)


def kernel(x, W_fw, b_fw, W_bw, b_bw, t_steps=None, trace=False):
    global LAST_EXEC_NS, LAST_TRACE
    if trace:
        _ensure_trace_hook()
    from concourse.bass_utils import run_bass_kernel_spmd

    x = np.asarray(x, dtype=np.float32)
    ts = t_steps or x.shape[1]
    nc = _get_program(ts)

    # block-diagonal idb: [:, q, 32q+b] = e_b rows + bias row of ones
    idb = np.zeros((33, 4, 128), np.float32)
    for q in range(4):
        idb[:32, q, 32 * q:32 * (q + 1)] = np.eye(32, dtype=np.float32)
        idb[32, q, 32 * q:32 * (q + 1)] = 1.0

    Wf, bf = _permute_gates(np.asarray(W_fw, np.float32), np.asarray(b_fw, np.float32))
    Wb, bb = _permute_gates(np.asarray(W_bw, np.float32), np.asarray(b_bw, np.float32))

    x_rev = np.ascontiguousarray(x[:, ::-1])

    core0 = {"x": x, "Wx": np.ascontiguousarray(Wf[:D]),
             "Wh": np.ascontiguousarray(Wf[D:]), "bv": bf, "idb": idb}
    core1 = {"x": x_rev, "Wx": np.ascontiguousarray(Wb[:D]),
             "Wh": np.ascontiguousarray(Wb[D:]), "bv": bb, "idb": idb}
    in_maps = [core0, core1] + [core0] * (N_CORES - 2)

    res = run_bass_kernel_spmd(nc, in_maps, list(range(N_CORES)), trace=trace)
    if res.exec_time_ns is not None:
        LAST_EXEC_NS = res.exec_time_ns
    if res.instructions_and_trace is not None:
        LAST_TRACE = res.instructions_and_trace[1]

    h_fw = res.results[0]["out_h"].transpose(2, 0, 1)          # [B, T, H]
    h_bw = res.results[1]["out_h"][::-1].transpose(2, 0, 1)
    return np.ascontiguousarray(
        np.concatenate([h_fw, h_bw], axis=-1).astype(np.float32))


# revision 18
# speedup vs baseline: 1.0056x; 1.0056x over previous
"""Bidirectional LSTM (B=32, T=512, D=H=512) on 8 Trainium2 NeuronCores.

Strategy:
  - One SPMD program on all 8 cores. Core 0 runs the forward direction,
    core 1 runs the backward direction (same instruction stream, fed
    time-reversed x and the bw weights). Other cores run the same program
    on replicated data; their outputs are ignored.
  - Per step, z is computed into ONE [128, 512] PSUM bank: partition
    32q+b holds the gates for batch row b, hidden chunk q (columns
    [f_q | i_q | o_q | g_q], 4x128). Each contraction matmul (k, q) uses
    a [128, 128] stationary whose only non-zero columns are block q
    (a sliding 4-slot window over a zero-padded arena holding hT_k), so
    all 20 matmuls are standard full-width M=128 accumulations into the
    same bank. The batch-stacked layout lets the elementwise tail run on
    all 128 partitions (3 ACT + 3 DVE ops) instead of 32.
  - h [128,128] (batch+chunk stacked) is PE-transposed per 32-row block
    into hT [128, 4, 32] and copied into the arena's center slots.
  - xp = x @ Wx + b is precomputed: the first T-chunk as a prologue,
    later chunks interleaved into the recurrence steps so the PE stays
    busy during each step's ACT/DVE tail.
  - Output is written as [T, H, B] per direction and reassembled on host.
"""

import os
import sys
import numpy as np

for _p in ("/opt/trn_rl_repo", "/root/.axon_site/_ro/trn_rl_repo"):
    if os.path.isdir(_p) and _p not in sys.path:
        sys.path.insert(0, _p)

B, T, D, H = 32, 512, 512, 512
G = 4 * H
N_CORES = 8

_PROG_CACHE = {}


def _build_program(t_steps=T, reps=1):
    from contextlib import ExitStack
    import concourse.bacc as bacc
    import concourse.tile as tile
    import concourse.mybir as mybir
    from concourse import masks

    f32 = mybir.dt.float32
    f32r = mybir.dt.float32r
    AF = mybir.ActivationFunctionType

    nc = bacc.Bacc("TRN2", target_bir_lowering=False, debug=False,
                   num_devices=N_CORES)

    x_t = nc.dram_tensor("x", [B, t_steps, D], f32, kind="ExternalInput")
    Wx_t = nc.dram_tensor("Wx", [D, G], f32r, kind="ExternalInput")
    Wh_t = nc.dram_tensor("Wh", [H, G], f32r, kind="ExternalInput")
    bv_t = nc.dram_tensor("bv", [G], f32r, kind="ExternalInput")
    idb_t = nc.dram_tensor("idb", [33, 4, 128], f32r, kind="ExternalInput")
    out_t = nc.dram_tensor("out_h", [t_steps, H, B], f32r, kind="ExternalOutput")

    TCH = min(128, t_steps)
    TC = t_steps // TCH
    HH = H // 4  # 128: hidden chunk size

    with tile.TileContext(nc) as tc, ExitStack() as ctx:
        wpool = ctx.enter_context(tc.tile_pool(name="w", bufs=1))
        tpool = ctx.enter_context(tc.tile_pool(name="tmp", bufs=3))
        xpool = ctx.enter_context(tc.tile_pool(name="xin", bufs=2))
        ppool = ctx.enter_context(tc.tile_pool(name="ps", bufs=3, space="PSUM"))
        fpool = ctx.enter_context(tc.tile_pool(name="fill", bufs=1, space="PSUM"))
        tppool = ctx.enter_context(tc.tile_pool(name="tps", bufs=2, space="PSUM"))
        p1pool = ctx.enter_context(tc.tile_pool(name="p1s", bufs=1, space="PSUM"))
        dpool = ctx.enter_context(tc.tile_pool(name="dram", bufs=1, space="DRAM"))

        for _rep in range(reps):
            ident = wpool.tile([128, 128], f32)
            masks.make_identity(nc, ident[:])

            idb_sb = wpool.tile([33, 4, 128], f32r)
            nc.sync.dma_start(idb_sb[:], idb_t.ap())

            Wx_sb = wpool.tile([128, 4, G], f32r, tag="Wbig")
            for k in range(4):
                nc.sync.dma_start(Wx_sb[:, k, :], Wx_t.ap()[k * 128:(k + 1) * 128, :])
            Wh_sb = wpool.tile([128, 4, G], f32r, tag="Wbig2")
            for k in range(4):
                nc.sync.dma_start(Wh_sb[:, k, :], Wh_t.ap()[k * 128:(k + 1) * 128, :])

            xp_dram = dpool.tile([t_steps, B, G], f32r)

            # ---- phase-1 quarter-m-tile quantum emitter -------------------
            p1_state = {}

            def emit_p1_quarter(tcki, b, n):
                tsl = slice(tcki * TCH, (tcki + 1) * TCH)
                if n == 0:
                    xt = xpool.tile([TCH, D], f32, tag="xt")
                    nc.sync.dma_start(xt[:], x_t.ap()[b, tsl, :])
                    xT_ps = p1pool.tile([128, 4, TCH], f32, tag="p1")
                    for k in range(4):
                        nc.tensor.transpose(xT_ps[:, k, :],
                                            xt[:, k * 128:(k + 1) * 128],
                                            ident[0:TCH, 0:TCH])
                    xT_sb = xpool.tile([128, 4, TCH], f32r, tag="xT")
                    nc.vector.tensor_copy(xT_sb[:], xT_ps[:])
                    zx = xpool.tile([TCH, G], f32r, tag="zx")
                    p1_state["xT"] = xT_sb
                    p1_state["zx"] = zx
                xT_sb = p1_state["xT"]
                zx = p1_state["zx"]
                zq = p1pool.tile([TCH, 512], f32, tag="p1")
                for k in range(4):
                    nc.tensor.matmul(zq[:], xT_sb[:, k, :],
                                     Wx_sb[:, k, n * 512:(n + 1) * 512],
                                     start=(k == 0), stop=(k == 3))
                nc.vector.tensor_copy(zx[:, n * 512:(n + 1) * 512], zq[:])
                if n == 3:
                    nc.sync.dma_start(xp_dram[tsl, b, :], zx[:])

            # quarter schedule: chunk 0 in the prologue; chunk c>0 spread
            # ~1 quarter/step over steps [TCH*(c-1), TCH*c - 6) so the PE
            # stays busy (HAM-warm) during every step's ACT/DVE tail.
            step_quanta = {}
            span = TCH - 6
            for c in range(1, TC):
                for j in range(4 * B):
                    st = TCH * (c - 1) + (j * span) // (4 * B)
                    step_quanta.setdefault(st, []).append((c, j // 4, j % 4))

            for b in range(B):
                for n in range(4):
                    emit_p1_quarter(0, b, n)

            # ---------------- recurrence ------------------------------------
            # z layout: [128, 512] — partition 32q+b, free [f|i|o|g] x 128
            RING = 6
            PREF = 3  # xr prefetch distance in steps
            xr = wpool.tile([33, RING, G], f32r, tag="xr")
            for s in range(RING):
                nc.sync.dma_start(xr[32:33, s, :], bv_t.ap()[None, :])

            # zero-padded stationary arena: per k, slots [0..6], hT_k at
            # slot 3; stationary for (k, q) = arena[:, k, 3-q:7-q, :]
            arena = wpool.tile([128, 4, 7, B], f32r, tag="arena")
            nc.vector.memset(arena[:].bitcast(f32), 0.0)
            # persistent [c | tanh_g] tile; c lives in [:, 0:HH]
            tgc = wpool.tile([128, 2 * HH], f32, tag="tgc")
            nc.vector.memset(tgc[:], 0.0)

            def emit_idb(t, zp):
                s = t % RING
                for q in range(4):
                    nc.tensor.matmul(zp[:], idb_sb[:, q, :],
                                     xr[:, s, 512 * q:512 * (q + 1)],
                                     start=(q == 0), stop=False)

            # prefetch first xr slots
            for t0 in range(min(PREF, t_steps)):
                nc.sync.dma_start(xr[0:32, t0 % RING, :], xp_dram[t0, :, :])

            zp = ppool.tile([128, 512], f32, tag="z")
            emit_idb(0, zp)

            for t in range(t_steps):
                tp = t + PREF
                if tp < t_steps:
                    nc.sync.dma_start(xr[0:32, tp % RING, :], xp_dram[tp, :, :])

                # contraction: for each k, 4 block-MMs (zero-padded cols)
                for k in range(4):
                    for q in range(4):
                        nc.tensor.matmul(zp[:],
                                         arena[:, k, 3 - q:7 - q, :],
                                         Wh_sb[:, k, 512 * q:512 * (q + 1)],
                                         start=False,
                                         stop=(k == 3 and q == 3))

                # dependency-free filler matmuls: start the instant the
                # z-block ends (ahead of the possibly-waiting idb round in
                # the in-order PE queue), so the PE never idles long enough
                # for the HAM clock gate to re-throttle to 1.2 GHz.
                quanta = step_quanta.get(t, ())
                if t + 1 < t_steps:
                    warm = fpool.tile([128, 512], f32, tag="warm")
                    nfill = 2 if quanta else 4
                    for k in range(nfill):
                        nc.tensor.matmul(warm[:], Wx_sb[:, 0, 0:128],
                                         Wx_sb[:, 2, 0:512],
                                         start=(k == 0), stop=(k == nfill - 1))

                # idb round for t+1 fills more of the PE tail window
                if t + 1 < t_steps:
                    zp_next = ppool.tile([128, 512], f32, tag="z")
                    emit_idb(t + 1, zp_next)

                # --- tail (all ops on 128 partitions) ---
                sfi = tpool.tile([128, 2 * HH], f32, tag="sfi")
                nc.scalar.activation(sfi[:], zp[:, 0:2 * HH], AF.Sigmoid)
                nc.scalar.activation(tgc[:, HH:2 * HH], zp[:, 3 * HH:4 * HH],
                                     AF.Tanh)
                so = tpool.tile([128, HH], f32, tag="so")
                nc.scalar.activation(so[:], zp[:, 2 * HH:3 * HH], AF.Sigmoid)

                t12 = tpool.tile([128, 2 * HH], f32, tag="t12")
                nc.vector.tensor_mul(t12[:], sfi[:], tgc[:])
                nc.vector.tensor_add(tgc[:, 0:HH], t12[:, 0:HH], t12[:, HH:2 * HH])
                tcl = tpool.tile([128, HH], f32, tag="tc")
                nc.scalar.activation(tcl[:], tgc[:, 0:HH], AF.Tanh)
                hs = tpool.tile([128, HH], f32, tag="hs")
                nc.vector.tensor_mul(hs[:], so[:], tcl[:])

                # transpose h into stationary layout, write arena center slots
                hT_ps = tppool.tile([128, 128], f32, tag="tp")
                nc.tensor.transpose(hT_ps[:], hs[:], ident[:])
                nc.vector.tensor_copy(
                    arena[:, :, 3, :],
                    hT_ps[:].rearrange("p (k b) -> p k b", k=4))

                nc.sync.dma_start(out_t.ap()[t].rearrange("(k p) b -> p k b", p=128),
                                  arena[:, :, 3, :])

                for (c, bq, nq) in quanta:
                    emit_p1_quarter(c, bq, nq)

                if t + 1 < t_steps:
                    zp = zp_next

    nc.compile()
    return nc


def _get_program(t_steps=T, reps=1):
    key = (t_steps, reps)
    if key not in _PROG_CACHE:
        _PROG_CACHE[key] = _build_program(t_steps, reps)
    return _PROG_CACHE[key]


def _permute_gates(W, b):
    # reference gate order [i, f, o, g] (each H wide) -> kernel column order
    # per hidden chunk q (128 wide): [f_q | i_q | o_q | g_q]
    i_, f_, o_, g_ = (W[:, k * H:(k + 1) * H] for k in range(4))
    ib, fb, ob, gb = (b[k * H:(k + 1) * H] for k in range(4))
    HH = H // 4
    wcols, bcols = [], []
    for q in range(4):
        sl = slice(q * HH, (q + 1) * HH)
        wcols += [f_[:, sl], i_[:, sl], o_[:, sl], g_[:, sl]]
        bcols += [fb[sl], ib[sl], ob[sl], gb[sl]]
    return (np.ascontiguousarray(np.concatenate(wcols, axis=1)),
            np.ascontiguousarray(np.concatenate(bcols)))


LAST_EXEC_NS = None
LAST_TRACE = None


def _ensure_trace_hook():
    # The container's antenv stub lacks axon_hooks, so trn_boot's NTFF
    # profile hook registration silently degraded. Recreate it here.
    import types
    if "antenv.axon_hooks" in sys.modules:
        return
    mod = types.ModuleType("antenv.axon_hooks")
    _h = [None]
    mod.set_axon_ntff_profile_hook = lambda h: _h.__setitem__(0, h)
    mod.get_axon_ntff_profile_hook = lambda: _h[0]
    sys.modules["antenv.axon_hooks"] = mod
    import antenv
    antenv.axon_hooks = mod
    try:
        from trn_agent_boot.trn_boot import _ntff_profile_via_ctypes
        mod.set_axon_ntff_profile_hook(
            _ntff_profile_via_ctypes("/opt/axon/libaxon_pjrt.so"))
    except Exception as e:
        print(f"trace hook setup failed: {e}", file=sys.stderr)


def kernel(x, W_fw, b_fw, W_bw, b_bw, t_steps=None, trace=False):
    global LAST_EXEC_NS, LAST_TRACE
    if trace:
        _ensure_trace_hook()
    from concourse.bass_utils import run_bass_kernel_spmd

    x = np.asarray(x, dtype=np.float32)
    ts = t_steps or x.shape[1]
    nc = _get_program(ts)

    # block-diagonal idb: [:, q, 32q+b] = e_b rows + bias row of ones
    idb = np.zeros((33, 4, 128), np.float32)
    for q in range(4):
        idb[:32, q, 32 * q:32 * (q + 1)] = np.eye(32, dtype=np.float32)
        idb[32, q, 32 * q:32 * (q + 1)] = 1.0

    Wf, bf = _permute_gates(np.asarray(W_fw, np.float32), np.asarray(b_fw, np.float32))
    Wb, bb = _permute_gates(np.asarray(W_bw, np.float32), np.asarray(b_bw, np.float32))

    x_rev = np.ascontiguousarray(x[:, ::-1])

    core0 = {"x": x, "Wx": np.ascontiguousarray(Wf[:D]),
             "Wh": np.ascontiguousarray(Wf[D:]), "bv": bf, "idb": idb}
    core1 = {"x": x_rev, "Wx": np.ascontiguousarray(Wb[:D]),
             "Wh": np.ascontiguousarray(Wb[D:]), "bv": bb, "idb": idb}
    in_maps = [core0, core1] + [core0] * (N_CORES - 2)

    res = run_bass_kernel_spmd(nc, in_maps, list(range(N_CORES)), trace=trace)
    if res.exec_time_ns is not None:
        LAST_EXEC_NS = res.exec_time_ns
    if res.instructions_and_trace is not None:
        LAST_TRACE = res.instructions_and_trace[1]

    h_fw = res.results[0]["out_h"].transpose(2, 0, 1)          # [B, T, H]
    h_bw = res.results[1]["out_h"][::-1].transpose(2, 0, 1)
    return np.ascontiguousarray(
        np.concatenate([h_fw, h_bw], axis=-1).astype(np.float32))


# revision 22
# speedup vs baseline: 1.0232x; 1.0174x over previous
"""Bidirectional LSTM (B=32, T=512, D=H=512) on 8 Trainium2 NeuronCores.

Strategy:
  - One SPMD program on all 8 cores. Core 0 runs the forward direction,
    core 1 runs the backward direction (same instruction stream, fed
    time-reversed x and the bw weights). Other cores run the same program
    on replicated data; their outputs are ignored.
  - Per step, z is computed into ONE [128, 512] PSUM bank: partition
    32q+b holds the gates for batch row b, hidden chunk q (columns
    [f_q | i_q | o_q | g_q], 4x128). Each contraction matmul (k, q) uses
    a [128, 128] stationary whose only non-zero columns are block q
    (a sliding 4-slot window over a zero-padded arena holding hT_k), so
    all 20 matmuls are standard full-width M=128 accumulations into the
    same bank. The batch-stacked layout lets the elementwise tail run on
    all 128 partitions (3 ACT + 3 DVE ops) instead of 32.
  - h [128,128] (batch+chunk stacked) is PE-transposed per 32-row block
    into hT [128, 4, 32] and copied into the arena's center slots.
  - xp = x @ Wx + b is precomputed: the first T-chunk as a prologue,
    later chunks interleaved into the recurrence steps so the PE stays
    busy during each step's ACT/DVE tail.
  - Output is written as [T, H, B] per direction and reassembled on host.
"""

import os
import sys
import numpy as np

for _p in ("/opt/trn_rl_repo", "/root/.axon_site/_ro/trn_rl_repo"):
    if os.path.isdir(_p) and _p not in sys.path:
        sys.path.insert(0, _p)

B, T, D, H = 32, 512, 512, 512
G = 4 * H
N_CORES = 8

_PROG_CACHE = {}


def _build_program(t_steps=T, reps=1):
    from contextlib import ExitStack
    import concourse.bacc as bacc
    import concourse.tile as tile
    import concourse.mybir as mybir
    from concourse import masks

    f32 = mybir.dt.float32
    f32r = mybir.dt.float32r
    AF = mybir.ActivationFunctionType

    nc = bacc.Bacc("TRN2", target_bir_lowering=False, debug=False,
                   num_devices=N_CORES)

    x_t = nc.dram_tensor("x", [B, t_steps, D], f32, kind="ExternalInput")
    Wx_t = nc.dram_tensor("Wx", [D, G], f32r, kind="ExternalInput")
    Wh_t = nc.dram_tensor("Wh", [H, G], f32r, kind="ExternalInput")
    bv_t = nc.dram_tensor("bv", [G], f32r, kind="ExternalInput")
    idb_t = nc.dram_tensor("idb", [33, 4, 128], f32r, kind="ExternalInput")
    out_t = nc.dram_tensor("out_h", [t_steps, H, B], f32r, kind="ExternalOutput")

    TCH = min(128, t_steps)
    TC = t_steps // TCH
    HH = H // 4  # 128: hidden chunk size

    with tile.TileContext(nc) as tc, ExitStack() as ctx:
        wpool = ctx.enter_context(tc.tile_pool(name="w", bufs=1))
        tpool = ctx.enter_context(tc.tile_pool(name="tmp", bufs=3))
        xpool = ctx.enter_context(tc.tile_pool(name="xin", bufs=2))
        ppool = ctx.enter_context(tc.tile_pool(name="ps", bufs=3, space="PSUM"))
        fpool = ctx.enter_context(tc.tile_pool(name="fill", bufs=1, space="PSUM"))
        tppool = ctx.enter_context(tc.tile_pool(name="tps", bufs=2, space="PSUM"))
        p1pool = ctx.enter_context(tc.tile_pool(name="p1s", bufs=1, space="PSUM"))
        dpool = ctx.enter_context(tc.tile_pool(name="dram", bufs=1, space="DRAM"))

        for _rep in range(reps):
            ident = wpool.tile([128, 128], f32)
            masks.make_identity(nc, ident[:])

            idb_sb = wpool.tile([33, 4, 128], f32r)
            nc.sync.dma_start(idb_sb[:], idb_t.ap())

            Wx_sb = wpool.tile([128, 4, G], f32r, tag="Wbig")
            for k in range(4):
                nc.sync.dma_start(Wx_sb[:, k, :], Wx_t.ap()[k * 128:(k + 1) * 128, :])
            Wh_sb = wpool.tile([128, 4, G], f32r, tag="Wbig2")
            for k in range(4):
                nc.sync.dma_start(Wh_sb[:, k, :], Wh_t.ap()[k * 128:(k + 1) * 128, :])

            xp_dram = dpool.tile([t_steps, B, G], f32r)

            # ---- phase-1 quarter-m-tile quantum emitter -------------------
            p1_state = {}

            def emit_p1_quarter(tcki, b, n):
                tsl = slice(tcki * TCH, (tcki + 1) * TCH)
                if n == 0:
                    xt = xpool.tile([TCH, D], f32, tag="xt")
                    nc.sync.dma_start(xt[:], x_t.ap()[b, tsl, :])
                    xT_ps = p1pool.tile([128, 4, TCH], f32, tag="p1")
                    for k in range(4):
                        nc.tensor.transpose(xT_ps[:, k, :],
                                            xt[:, k * 128:(k + 1) * 128],
                                            ident[0:TCH, 0:TCH])
                    xT_sb = xpool.tile([128, 4, TCH], f32r, tag="xT")
                    nc.vector.tensor_copy(xT_sb[:], xT_ps[:])
                    zx = xpool.tile([TCH, G], f32r, tag="zx")
                    p1_state["xT"] = xT_sb
                    p1_state["zx"] = zx
                xT_sb = p1_state["xT"]
                zx = p1_state["zx"]
                zq = p1pool.tile([TCH, 512], f32, tag="p1")
                for k in range(4):
                    nc.tensor.matmul(zq[:], xT_sb[:, k, :],
                                     Wx_sb[:, k, n * 512:(n + 1) * 512],
                                     start=(k == 0), stop=(k == 3))
                nc.vector.tensor_copy(zx[:, n * 512:(n + 1) * 512], zq[:])
                if n == 3:
                    nc.sync.dma_start(xp_dram[tsl, b, :], zx[:])

            # quarter schedule: chunk 0 in the prologue; chunk c>0 spread
            # ~1 quarter/step over steps [TCH*(c-1), TCH*c - 6) so the PE
            # stays busy (HAM-warm) during every step's ACT/DVE tail.
            step_quanta = {}
            span = TCH - 6
            for c in range(1, TC):
                for j in range(4 * B):
                    st = TCH * (c - 1) + (j * span) // (4 * B)
                    step_quanta.setdefault(st, []).append((c, j // 4, j % 4))

            for b in range(B):
                for n in range(4):
                    emit_p1_quarter(0, b, n)

            # ---------------- recurrence ------------------------------------
            # z layout: [128, 512] — partition 32q+b, free [f|i|o|g] x 128
            RING = 6
            PREF = 3  # xr prefetch distance in steps
            xr = wpool.tile([33, RING, G], f32r, tag="xr")
            for s in range(RING):
                nc.sync.dma_start(xr[32:33, s, :], bv_t.ap()[None, :])

            # zero-padded stationary arena: per k, slots [0..6], hT_k at
            # slot 3; stationary for (k, q) = arena[:, k, 3-q:7-q, :]
            arena = wpool.tile([128, 4, 7, B], f32r, tag="arena")
            nc.vector.memset(arena[:].bitcast(f32), 0.0)
            # persistent [c | tanh_g] tile; c lives in [:, 0:HH]
            tgc = wpool.tile([128, 2 * HH], f32, tag="tgc")
            nc.vector.memset(tgc[:], 0.0)

            def emit_idb(t, zp):
                s = t % RING
                for q in range(4):
                    nc.tensor.matmul(zp[:], idb_sb[:, q, :],
                                     xr[:, s, 512 * q:512 * (q + 1)],
                                     start=(q == 0), stop=False)

            # prefetch first xr slots
            for t0 in range(min(PREF, t_steps)):
                nc.sync.dma_start(xr[0:32, t0 % RING, :], xp_dram[t0, :, :])

            warm_ps = fpool.tile([128, 512], f32, tag="warm")
            nc.tensor.matmul(warm_ps[:], Wx_sb[:, 0, 0:128], Wx_sb[:, 2, 0:512],
                             start=True, stop=False, skip_group_check=True)

            zp = ppool.tile([128, 512], f32, tag="z")
            emit_idb(0, zp)

            for t in range(t_steps):
                tp = t + PREF
                if tp < t_steps:
                    nc.sync.dma_start(xr[0:32, tp % RING, :], xp_dram[tp, :, :])

                # contraction: for each k, 4 block-MMs (zero-padded cols)
                for k in range(4):
                    for q in range(4):
                        nc.tensor.matmul(zp[:],
                                         arena[:, k, 3 - q:7 - q, :],
                                         Wh_sb[:, k, 512 * q:512 * (q + 1)],
                                         start=False,
                                         stop=(k == 3 and q == 3))

                # dependency-free filler matmuls: start the instant the
                # z-block ends (ahead of the possibly-waiting idb round in
                # the in-order PE queue), so the PE never idles long enough
                # for the HAM clock gate to re-throttle to 1.2 GHz. All
                # fillers accumulate into one PSUM group that is read once
                # after the loop, so they survive dead-code elimination.
                quanta = step_quanta.get(t, ())
                if t + 1 < t_steps:
                    nfill = 2 if quanta else 4
                    for k in range(nfill):
                        nc.tensor.matmul(warm_ps[:], Wx_sb[:, 0, 0:128],
                                         Wx_sb[:, 2, 0:512],
                                         start=False, stop=False,
                                         skip_group_check=True)

                # idb round for t+1 fills more of the PE tail window
                if t + 1 < t_steps:
                    zp_next = ppool.tile([128, 512], f32, tag="z")
                    emit_idb(t + 1, zp_next)

                # --- tail (all ops on 128 partitions) ---
                sfi = tpool.tile([128, 2 * HH], f32, tag="sfi")
                nc.scalar.activation(sfi[:], zp[:, 0:2 * HH], AF.Sigmoid)
                nc.scalar.activation(tgc[:, HH:2 * HH], zp[:, 3 * HH:4 * HH],
                                     AF.Tanh)
                so = tpool.tile([128, HH], f32, tag="so")
                nc.scalar.activation(so[:], zp[:, 2 * HH:3 * HH], AF.Sigmoid)

                t12 = tpool.tile([128, 2 * HH], f32, tag="t12")
                nc.vector.tensor_mul(t12[:], sfi[:], tgc[:])
                nc.vector.tensor_add(tgc[:, 0:HH], t12[:, 0:HH], t12[:, HH:2 * HH])
                tcl = tpool.tile([128, HH], f32, tag="tc")
                nc.scalar.activation(tcl[:], tgc[:, 0:HH], AF.Tanh)
                hs = tpool.tile([128, HH], f32, tag="hs")
                nc.vector.tensor_mul(hs[:], so[:], tcl[:])

                # transpose h into stationary layout, write arena center slots
                hT_ps = tppool.tile([128, 128], f32, tag="tp")
                nc.tensor.transpose(hT_ps[:], hs[:], ident[:])
                nc.vector.tensor_copy(
                    arena[:, :, 3, :],
                    hT_ps[:].rearrange("p (k b) -> p k b", k=4))

                nc.sync.dma_start(out_t.ap()[t].rearrange("(k p) b -> p k b", p=128),
                                  arena[:, :, 3, :])

                for (c, bq, nq) in quanta:
                    emit_p1_quarter(c, bq, nq)

                if t + 1 < t_steps:
                    zp = zp_next

            # close + anchor the filler accumulation group
            nc.tensor.matmul(warm_ps[:], Wx_sb[:, 0, 0:128], Wx_sb[:, 2, 0:512],
                             start=False, stop=True, skip_group_check=True)
            warm_sb = wpool.tile([1, 512], f32, tag="warmsb")
            nc.vector.tensor_copy(warm_sb[:], warm_ps[0:1, :])
            nc.sync.dma_start(xp_dram[0, 0:1, 0:512].bitcast(f32), warm_sb[:])

    nc.compile()
    return nc


def _get_program(t_steps=T, reps=1):
    key = (t_steps, reps)
    if key not in _PROG_CACHE:
        _PROG_CACHE[key] = _build_program(t_steps, reps)
    return _PROG_CACHE[key]


def _permute_gates(W, b):
    # reference gate order [i, f, o, g] (each H wide) -> kernel column order
    # per hidden chunk q (128 wide): [f_q | i_q | o_q | g_q]
    i_, f_, o_, g_ = (W[:, k * H:(k + 1) * H] for k in range(4))
    ib, fb, ob, gb = (b[k * H:(k + 1) * H] for k in range(4))
    HH = H // 4
    wcols, bcols = [], []
    for q in range(4):
        sl = slice(q * HH, (q + 1) * HH)
        wcols += [f_[:, sl], i_[:, sl], o_[:, sl], g_[:, sl]]
        bcols += [fb[sl], ib[sl], ob[sl], gb[sl]]
    return (np.ascontiguousarray(np.concatenate(wcols, axis=1)),
            np.ascontiguousarray(np.concatenate(bcols)))


LAST_EXEC_NS = None
LAST_TRACE = None


def _ensure_trace_hook():
    # The container's antenv stub lacks axon_hooks, so trn_boot's NTFF
    # profile hook registration silently degraded. Recreate it here.
    import types
    if "antenv.axon_hooks" in sys.modules:
        return
    mod = types.ModuleType("antenv.axon_hooks")
    _h = [None]
    mod.set_axon_ntff_profile_hook = lambda h: _h.__setitem__(0, h)
    mod.get_axon_ntff_profile_hook = lambda: _h[0]
    sys.modules["antenv.axon_hooks"] = mod
    import antenv
    antenv.axon_hooks = mod
    try:
        from trn_agent_boot.trn_boot import _ntff_profile_via_ctypes
        mod.set_axon_ntff_profile_hook(
            _ntff_profile_via_ctypes("/opt/axon/libaxon_pjrt.so"))
    except Exception as e:
        print(f"trace hook setup failed: {e}", file=sys.stderr)


def kernel(x, W_fw, b_fw, W_bw, b_bw, t_steps=None, trace=False):
    global LAST_EXEC_NS, LAST_TRACE
    if trace:
        _ensure_trace_hook()
    from concourse.bass_utils import run_bass_kernel_spmd

    x = np.asarray(x, dtype=np.float32)
    ts = t_steps or x.shape[1]
    nc = _get_program(ts)

    # block-diagonal idb: [:, q, 32q+b] = e_b rows + bias row of ones
    idb = np.zeros((33, 4, 128), np.float32)
    for q in range(4):
        idb[:32, q, 32 * q:32 * (q + 1)] = np.eye(32, dtype=np.float32)
        idb[32, q, 32 * q:32 * (q + 1)] = 1.0

    Wf, bf = _permute_gates(np.asarray(W_fw, np.float32), np.asarray(b_fw, np.float32))
    Wb, bb = _permute_gates(np.asarray(W_bw, np.float32), np.asarray(b_bw, np.float32))

    x_rev = np.ascontiguousarray(x[:, ::-1])

    core0 = {"x": x, "Wx": np.ascontiguousarray(Wf[:D]),
             "Wh": np.ascontiguousarray(Wf[D:]), "bv": bf, "idb": idb}
    core1 = {"x": x_rev, "Wx": np.ascontiguousarray(Wb[:D]),
             "Wh": np.ascontiguousarray(Wb[D:]), "bv": bb, "idb": idb}
    in_maps = [core0, core1] + [core0] * (N_CORES - 2)

    res = run_bass_kernel_spmd(nc, in_maps, list(range(N_CORES)), trace=trace)
    if res.exec_time_ns is not None:
        LAST_EXEC_NS = res.exec_time_ns
    if res.instructions_and_trace is not None:
        LAST_TRACE = res.instructions_and_trace[1]

    h_fw = res.results[0]["out_h"].transpose(2, 0, 1)          # [B, T, H]
    h_bw = res.results[1]["out_h"][::-1].transpose(2, 0, 1)
    return np.ascontiguousarray(
        np.concatenate([h_fw, h_bw], axis=-1).astype(np.float32))


# revision 24
# speedup vs baseline: 1.0701x; 1.0459x over previous
"""Bidirectional LSTM (B=32, T=512, D=H=512) on 8 Trainium2 NeuronCores.

Strategy:
  - One SPMD program on all 8 cores. Core 0 runs the forward direction,
    core 1 runs the backward direction (same instruction stream, fed
    time-reversed x and the bw weights). Other cores run the same program
    on replicated data; their outputs are ignored.
  - Per step, z is computed into ONE [128, 512] PSUM bank: partition
    32q+b holds the gates for batch row b, hidden chunk q (columns
    [f_q | i_q | o_q | g_q], 4x128). Each contraction matmul (k, q) uses
    a [128, 128] stationary whose only non-zero columns are block q
    (a sliding 4-slot window over a zero-padded arena holding hT_k), so
    all 20 matmuls are standard full-width M=128 accumulations into the
    same bank. The batch-stacked layout lets the elementwise tail run on
    all 128 partitions (3 ACT + 3 DVE ops) instead of 32.
  - h [128,128] (batch+chunk stacked) is PE-transposed per 32-row block
    into hT [128, 4, 32] and copied into the arena's center slots.
  - xp = x @ Wx + b is precomputed: the first T-chunk as a prologue,
    later chunks interleaved into the recurrence steps so the PE stays
    busy during each step's ACT/DVE tail.
  - Output is written as [T, H, B] per direction and reassembled on host.
"""

import os
import sys
import numpy as np

for _p in ("/opt/trn_rl_repo", "/root/.axon_site/_ro/trn_rl_repo"):
    if os.path.isdir(_p) and _p not in sys.path:
        sys.path.insert(0, _p)

B, T, D, H = 32, 512, 512, 512
G = 4 * H
N_CORES = 8

_PROG_CACHE = {}


def _build_program(t_steps=T, reps=1):
    from contextlib import ExitStack
    import concourse.bacc as bacc
    import concourse.tile as tile
    import concourse.mybir as mybir
    from concourse import masks

    f32 = mybir.dt.float32
    f32r = mybir.dt.float32r
    AF = mybir.ActivationFunctionType

    nc = bacc.Bacc("TRN2", target_bir_lowering=False, debug=False,
                   num_devices=N_CORES)

    x_t = nc.dram_tensor("x", [B, t_steps, D], f32, kind="ExternalInput")
    Wx_t = nc.dram_tensor("Wx", [D, G], f32r, kind="ExternalInput")
    Wh_t = nc.dram_tensor("Wh", [H, G], f32r, kind="ExternalInput")
    bv_t = nc.dram_tensor("bv", [G], f32r, kind="ExternalInput")
    idb_t = nc.dram_tensor("idb", [33, 4, 128], f32r, kind="ExternalInput")
    out_t = nc.dram_tensor("out_h", [t_steps, H, B], f32r, kind="ExternalOutput")

    TCH = min(128, t_steps)
    TC = t_steps // TCH
    HH = H // 4  # 128: hidden chunk size

    with tile.TileContext(nc) as tc, ExitStack() as ctx:
        wpool = ctx.enter_context(tc.tile_pool(name="w", bufs=1))
        tpool = ctx.enter_context(tc.tile_pool(name="tmp", bufs=3))
        xpool = ctx.enter_context(tc.tile_pool(name="xin", bufs=2))
        ppool = ctx.enter_context(tc.tile_pool(name="ps", bufs=4, space="PSUM"))
        fpool = ctx.enter_context(tc.tile_pool(name="fill", bufs=1, space="PSUM"))
        tppool = ctx.enter_context(tc.tile_pool(name="tps", bufs=2, space="PSUM"))
        p1pool = ctx.enter_context(tc.tile_pool(name="p1s", bufs=1, space="PSUM"))
        dpool = ctx.enter_context(tc.tile_pool(name="dram", bufs=1, space="DRAM"))

        for _rep in range(reps):
            ident = wpool.tile([128, 128], f32)
            masks.make_identity(nc, ident[:])

            idb_sb = wpool.tile([33, 4, 128], f32r)
            nc.sync.dma_start(idb_sb[:], idb_t.ap())

            Wx_sb = wpool.tile([128, 4, G], f32r, tag="Wbig")
            for k in range(4):
                nc.sync.dma_start(Wx_sb[:, k, :], Wx_t.ap()[k * 128:(k + 1) * 128, :])
            Wh_sb = wpool.tile([128, 4, G], f32r, tag="Wbig2")
            for k in range(4):
                nc.sync.dma_start(Wh_sb[:, k, :], Wh_t.ap()[k * 128:(k + 1) * 128, :])

            xp_dram = dpool.tile([t_steps, B, G], f32r)

            # ---- phase-1 quarter-m-tile quantum emitter -------------------
            p1_state = {}

            def emit_p1_quarter(tcki, b, n):
                tsl = slice(tcki * TCH, (tcki + 1) * TCH)
                if n == 0:
                    xt = xpool.tile([TCH, D], f32, tag="xt")
                    nc.sync.dma_start(xt[:], x_t.ap()[b, tsl, :])
                    xT_ps = p1pool.tile([128, 4, TCH], f32, tag="p1")
                    for k in range(4):
                        nc.tensor.transpose(xT_ps[:, k, :],
                                            xt[:, k * 128:(k + 1) * 128],
                                            ident[0:TCH, 0:TCH])
                    xT_sb = xpool.tile([128, 4, TCH], f32r, tag="xT")
                    nc.vector.tensor_copy(xT_sb[:], xT_ps[:])
                    zx = xpool.tile([TCH, G], f32r, tag="zx")
                    p1_state["xT"] = xT_sb
                    p1_state["zx"] = zx
                xT_sb = p1_state["xT"]
                zx = p1_state["zx"]
                zq = p1pool.tile([TCH, 512], f32, tag="p1")
                for k in range(4):
                    nc.tensor.matmul(zq[:], xT_sb[:, k, :],
                                     Wx_sb[:, k, n * 512:(n + 1) * 512],
                                     start=(k == 0), stop=(k == 3))
                nc.vector.tensor_copy(zx[:, n * 512:(n + 1) * 512], zq[:])
                if n == 3:
                    nc.sync.dma_start(xp_dram[tsl, b, :], zx[:])

            # quarter schedule: chunk 0 in the prologue; chunk c>0 spread
            # ~1 quarter/step over steps [TCH*(c-1), TCH*c - 6) so the PE
            # stays busy (HAM-warm) during every step's ACT/DVE tail.
            step_quanta = {}
            span = TCH - 6
            for c in range(1, TC):
                for j in range(4 * B):
                    st = TCH * (c - 1) + (j * span) // (4 * B)
                    step_quanta.setdefault(st, []).append((c, j // 4, j % 4))

            for b in range(B):
                for n in range(4):
                    emit_p1_quarter(0, b, n)

            # ---------------- recurrence ------------------------------------
            # z layout: [128, 512] — partition 32q+b, free [f|i|o|g] x 128
            RING = 6
            PREF = 3  # xr prefetch distance in steps
            xr = wpool.tile([33, RING, G], f32r, tag="xr")
            for s in range(RING):
                nc.sync.dma_start(xr[32:33, s, :], bv_t.ap()[None, :])

            # zero-padded stationary arena: per k, slots [0..6], hT_k at
            # slot 3; stationary for (k, q) = arena[:, k, 3-q:7-q, :]
            arena = wpool.tile([128, 4, 7, B], f32r, tag="arena")
            nc.vector.memset(arena[:].bitcast(f32), 0.0)
            # persistent [c | tanh_g] tile; c lives in [:, 0:HH]
            tgc = wpool.tile([128, 2 * HH], f32, tag="tgc")
            nc.vector.memset(tgc[:], 0.0)

            def emit_idb(t, zp):
                s = t % RING
                for q in range(4):
                    nc.tensor.matmul(zp[:], idb_sb[:, q, :],
                                     xr[:, s, 512 * q:512 * (q + 1)],
                                     start=(q == 0), stop=False)

            # prefetch first xr slots
            for t0 in range(min(PREF, t_steps)):
                nc.sync.dma_start(xr[0:32, t0 % RING, :], xp_dram[t0, :, :])

            warm_ps = fpool.tile([128, 512], f32, tag="warm")
            nc.tensor.matmul(warm_ps[:], Wx_sb[:, 0, 0:128], Wx_sb[:, 2, 0:512],
                             start=True, stop=False, skip_group_check=True)

            zp = ppool.tile([128, 512], f32, tag="z")
            emit_idb(0, zp)

            for t in range(t_steps):
                tp = t + PREF
                if tp < t_steps:
                    nc.sync.dma_start(xr[0:32, tp % RING, :], xp_dram[tp, :, :])

                # contraction: for each k, 4 block-MMs (zero-padded cols)
                for k in range(4):
                    for q in range(4):
                        nc.tensor.matmul(zp[:],
                                         arena[:, k, 3 - q:7 - q, :],
                                         Wh_sb[:, k, 512 * q:512 * (q + 1)],
                                         start=False,
                                         stop=(k == 3 and q == 3))

                # dependency-free filler matmuls: start the instant the
                # z-block ends (ahead of the possibly-waiting idb round in
                # the in-order PE queue), so the PE never idles long enough
                # for the HAM clock gate to re-throttle to 1.2 GHz. All
                # fillers accumulate into one PSUM group that is read once
                # after the loop, so they survive dead-code elimination.
                quanta = step_quanta.get(t, ())
                if t + 1 < t_steps:
                    nfill = 2 if quanta else 4
                    for k in range(nfill):
                        nc.tensor.matmul(warm_ps[:], Wx_sb[:, 0, 0:128],
                                         Wx_sb[:, 2, 0:512],
                                         start=False, stop=False,
                                         skip_group_check=True)

                # idb round for t+1 fills more of the PE tail window
                if t + 1 < t_steps:
                    zp_next = ppool.tile([128, 512], f32, tag="z")
                    emit_idb(t + 1, zp_next)

                # --- tail (all ops on 128 partitions) ---
                sfi = tpool.tile([128, 2 * HH], f32, tag="sfi")
                nc.scalar.activation(sfi[:], zp[:, 0:2 * HH], AF.Sigmoid)
                nc.scalar.activation(tgc[:, HH:2 * HH], zp[:, 3 * HH:4 * HH],
                                     AF.Tanh)
                so = tpool.tile([128, HH], f32, tag="so")
                nc.scalar.activation(so[:], zp[:, 2 * HH:3 * HH], AF.Sigmoid)

                t12 = tpool.tile([128, 2 * HH], f32, tag="t12")
                nc.vector.tensor_mul(t12[:], sfi[:], tgc[:])
                nc.vector.tensor_add(tgc[:, 0:HH], t12[:, 0:HH], t12[:, HH:2 * HH])
                tcl = tpool.tile([128, HH], f32, tag="tc")
                nc.scalar.activation(tcl[:], tgc[:, 0:HH], AF.Tanh)
                hs = tpool.tile([128, HH], f32, tag="hs")
                nc.vector.tensor_mul(hs[:], so[:], tcl[:])

                # transpose h into stationary layout, write arena center
                # slots; per-k copies so each k's matmul group of the next
                # step can start as soon as its arena segment lands
                hT_ps = tppool.tile([128, 128], f32, tag="tp")
                nc.tensor.transpose(hT_ps[:], hs[:], ident[:])
                for k in range(4):
                    nc.vector.tensor_copy(arena[:, k, 3, :],
                                          hT_ps[:, 32 * k:32 * (k + 1)])

                nc.sync.dma_start(out_t.ap()[t].rearrange("(k p) b -> p k b", p=128),
                                  arena[:, :, 3, :])

                for (c, bq, nq) in quanta:
                    emit_p1_quarter(c, bq, nq)

                if t + 1 < t_steps:
                    zp = zp_next

            # close + anchor the filler accumulation group
            nc.tensor.matmul(warm_ps[:], Wx_sb[:, 0, 0:128], Wx_sb[:, 2, 0:512],
                             start=False, stop=True, skip_group_check=True)
            warm_sb = wpool.tile([1, 512], f32, tag="warmsb")
            nc.vector.tensor_copy(warm_sb[:], warm_ps[0:1, :])
            nc.sync.dma_start(xp_dram[0, 0:1, 0:512].bitcast(f32), warm_sb[:])

    nc.compile()
    return nc


def _get_program(t_steps=T, reps=1):
    key = (t_steps, reps)
    if key not in _PROG_CACHE:
        _PROG_CACHE[key] = _build_program(t_steps, reps)
    return _PROG_CACHE[key]


def _permute_gates(W, b):
    # reference gate order [i, f, o, g] (each H wide) -> kernel column order
    # per hidden chunk q (128 wide): [f_q | i_q | o_q | g_q]
    i_, f_, o_, g_ = (W[:, k * H:(k + 1) * H] for k in range(4))
    ib, fb, ob, gb = (b[k * H:(k + 1) * H] for k in range(4))
    HH = H // 4
    wcols, bcols = [], []
    for q in range(4):
        sl = slice(q * HH, (q + 1) * HH)
        wcols += [f_[:, sl], i_[:, sl], o_[:, sl], g_[:, sl]]
        bcols += [fb[sl], ib[sl], ob[sl], gb[sl]]
    return (np.ascontiguousarray(np.concatenate(wcols, axis=1)),
            np.ascontiguousarray(np.concatenate(bcols)))


LAST_EXEC_NS = None
LAST_TRACE = None


def _ensure_trace_hook():
    # The container's antenv stub lacks axon_hooks, so trn_boot's NTFF
    # profile hook registration silently degraded. Recreate it here.
    import types
    if "antenv.axon_hooks" in sys.modules:
        return
    mod = types.ModuleType("antenv.axon_hooks")
    _h = [None]
    mod.set_axon_ntff_profile_hook = lambda h: _h.__setitem__(0, h)
    mod.get_axon_ntff_profile_hook = lambda: _h[0]
    sys.modules["antenv.axon_hooks"] = mod
    import antenv
    antenv.axon_hooks = mod
    try:
        from trn_agent_boot.trn_boot import _ntff_profile_via_ctypes
        mod.set_axon_ntff_profile_hook(
            _ntff_profile_via_ctypes("/opt/axon/libaxon_pjrt.so"))
    except Exception as e:
        print(f"trace hook setup failed: {e}", file=sys.stderr)


def kernel(x, W_fw, b_fw, W_bw, b_bw, t_steps=None, trace=False):
    global LAST_EXEC_NS, LAST_TRACE
    if trace:
        _ensure_trace_hook()
    from concourse.bass_utils import run_bass_kernel_spmd

    x = np.asarray(x, dtype=np.float32)
    ts = t_steps or x.shape[1]
    nc = _get_program(ts)

    # block-diagonal idb: [:, q, 32q+b] = e_b rows + bias row of ones
    idb = np.zeros((33, 4, 128), np.float32)
    for q in range(4):
        idb[:32, q, 32 * q:32 * (q + 1)] = np.eye(32, dtype=np.float32)
        idb[32, q, 32 * q:32 * (q + 1)] = 1.0

    Wf, bf = _permute_gates(np.asarray(W_fw, np.float32), np.asarray(b_fw, np.float32))
    Wb, bb = _permute_gates(np.asarray(W_bw, np.float32), np.asarray(b_bw, np.float32))

    x_rev = np.ascontiguousarray(x[:, ::-1])

    core0 = {"x": x, "Wx": np.ascontiguousarray(Wf[:D]),
             "Wh": np.ascontiguousarray(Wf[D:]), "bv": bf, "idb": idb}
    core1 = {"x": x_rev, "Wx": np.ascontiguousarray(Wb[:D]),
             "Wh": np.ascontiguousarray(Wb[D:]), "bv": bb, "idb": idb}
    in_maps = [core0, core1] + [core0] * (N_CORES - 2)

    res = run_bass_kernel_spmd(nc, in_maps, list(range(N_CORES)), trace=trace)
    if res.exec_time_ns is not None:
        LAST_EXEC_NS = res.exec_time_ns
    if res.instructions_and_trace is not None:
        LAST_TRACE = res.instructions_and_trace[1]

    h_fw = res.results[0]["out_h"].transpose(2, 0, 1)          # [B, T, H]
    h_bw = res.results[1]["out_h"][::-1].transpose(2, 0, 1)
    return np.ascontiguousarray(
        np.concatenate([h_fw, h_bw], axis=-1).astype(np.float32))


# revision 26
# speedup vs baseline: 1.0769x; 1.0064x over previous
"""Bidirectional LSTM (B=32, T=512, D=H=512) on 8 Trainium2 NeuronCores.

Strategy:
  - One SPMD program on all 8 cores. Core 0 runs the forward direction,
    core 1 runs the backward direction (same instruction stream, fed
    time-reversed x and the bw weights). Other cores run the same program
    on replicated data; their outputs are ignored.
  - Per step, z is computed into ONE [128, 512] PSUM bank: partition
    32q+b holds the gates for batch row b, hidden chunk q (columns
    [f_q | i_q | o_q | g_q], 4x128). Each contraction matmul (k, q) uses
    a [128, 128] stationary whose only non-zero columns are block q
    (a sliding 4-slot window over a zero-padded arena holding hT_k), so
    all 20 matmuls are standard full-width M=128 accumulations into the
    same bank. The batch-stacked layout lets the elementwise tail run on
    all 128 partitions (3 ACT + 3 DVE ops) instead of 32.
  - h [128,128] (batch+chunk stacked) is PE-transposed per 32-row block
    into hT [128, 4, 32] and copied into the arena's center slots.
  - xp = x @ Wx + b is precomputed: the first T-chunk as a prologue,
    later chunks interleaved into the recurrence steps so the PE stays
    busy during each step's ACT/DVE tail.
  - Output is written as [T, H, B] per direction and reassembled on host.
"""

import os
import sys
import numpy as np

for _p in ("/opt/trn_rl_repo", "/root/.axon_site/_ro/trn_rl_repo"):
    if os.path.isdir(_p) and _p not in sys.path:
        sys.path.insert(0, _p)

B, T, D, H = 32, 512, 512, 512
G = 4 * H
N_CORES = 8

_PROG_CACHE = {}


def _build_program(t_steps=T, reps=1):
    from contextlib import ExitStack
    import concourse.bacc as bacc
    import concourse.tile as tile
    import concourse.mybir as mybir
    from concourse import masks

    f32 = mybir.dt.float32
    f32r = mybir.dt.float32r
    AF = mybir.ActivationFunctionType

    nc = bacc.Bacc("TRN2", target_bir_lowering=False, debug=False,
                   num_devices=N_CORES)

    x_t = nc.dram_tensor("x", [B, t_steps, D], f32, kind="ExternalInput")
    Wx_t = nc.dram_tensor("Wx", [D, G], f32r, kind="ExternalInput")
    Wh_t = nc.dram_tensor("Wh", [H, G], f32r, kind="ExternalInput")
    bv_t = nc.dram_tensor("bv", [G], f32r, kind="ExternalInput")
    idb_t = nc.dram_tensor("idb", [33, 4, 128], f32r, kind="ExternalInput")
    out_t = nc.dram_tensor("out_h", [t_steps, H, B], f32r, kind="ExternalOutput")

    TCH = min(128, t_steps)
    TC = t_steps // TCH
    HH = H // 4  # 128: hidden chunk size

    with tile.TileContext(nc) as tc, ExitStack() as ctx:
        wpool = ctx.enter_context(tc.tile_pool(name="w", bufs=1))
        tpool = ctx.enter_context(tc.tile_pool(name="tmp", bufs=3))
        xpool = ctx.enter_context(tc.tile_pool(name="xin", bufs=2))
        ppool = ctx.enter_context(tc.tile_pool(name="ps", bufs=4, space="PSUM"))
        fpool = ctx.enter_context(tc.tile_pool(name="fill", bufs=1, space="PSUM"))
        tppool = ctx.enter_context(tc.tile_pool(name="tps", bufs=2, space="PSUM"))
        p1pool = ctx.enter_context(tc.tile_pool(name="p1s", bufs=1, space="PSUM"))
        dpool = ctx.enter_context(tc.tile_pool(name="dram", bufs=1, space="DRAM"))

        for _rep in range(reps):
            ident = wpool.tile([128, 128], f32)
            masks.make_identity(nc, ident[:])

            idb_sb = wpool.tile([33, 4, 128], f32r)
            nc.sync.dma_start(idb_sb[:], idb_t.ap())

            Wx_sb = wpool.tile([128, 4, G], f32r, tag="Wbig")
            for k in range(4):
                nc.sync.dma_start(Wx_sb[:, k, :], Wx_t.ap()[k * 128:(k + 1) * 128, :])
            Wh_sb = wpool.tile([128, 4, G], f32r, tag="Wbig2")
            for k in range(4):
                nc.sync.dma_start(Wh_sb[:, k, :], Wh_t.ap()[k * 128:(k + 1) * 128, :])

            xp_dram = dpool.tile([t_steps, B, G], f32r)

            # ---- phase-1 quarter-m-tile quantum emitter -------------------
            p1_state = {}

            def emit_p1_quarter(tcki, b, n):
                tsl = slice(tcki * TCH, (tcki + 1) * TCH)
                if n == 0:
                    xt = xpool.tile([TCH, D], f32, tag="xt")
                    nc.sync.dma_start(xt[:], x_t.ap()[b, tsl, :])
                    xT_ps = p1pool.tile([128, 4, TCH], f32, tag="p1")
                    for k in range(4):
                        nc.tensor.transpose(xT_ps[:, k, :],
                                            xt[:, k * 128:(k + 1) * 128],
                                            ident[0:TCH, 0:TCH])
                    xT_sb = xpool.tile([128, 4, TCH], f32r, tag="xT")
                    nc.vector.tensor_copy(xT_sb[:], xT_ps[:])
                    zx = xpool.tile([TCH, G], f32r, tag="zx")
                    p1_state["xT"] = xT_sb
                    p1_state["zx"] = zx
                xT_sb = p1_state["xT"]
                zx = p1_state["zx"]
                zq = p1pool.tile([TCH, 512], f32, tag="p1")
                for k in range(4):
                    nc.tensor.matmul(zq[:], xT_sb[:, k, :],
                                     Wx_sb[:, k, n * 512:(n + 1) * 512],
                                     start=(k == 0), stop=(k == 3))
                nc.vector.tensor_copy(zx[:, n * 512:(n + 1) * 512], zq[:])
                if n == 3:
                    nc.sync.dma_start(xp_dram[tsl, b, :], zx[:])

            # quarter schedule: chunk 0 in the prologue; chunk c>0 spread
            # ~1 quarter/step over steps [TCH*(c-1), TCH*c - 6) so the PE
            # stays busy (HAM-warm) during every step's ACT/DVE tail.
            step_quanta = {}
            span = TCH - 6
            for c in range(1, TC):
                for j in range(4 * B):
                    st = TCH * (c - 1) + (j * span) // (4 * B)
                    step_quanta.setdefault(st, []).append((c, j // 4, j % 4))

            for b in range(B):
                for n in range(4):
                    emit_p1_quarter(0, b, n)

            # ---------------- recurrence ------------------------------------
            # z layout: [128, 512] — partition 32q+b, free [f|i|o|g] x 128
            RING = 6
            PREF = 3  # xr prefetch distance in steps
            xr = wpool.tile([33, RING, G], f32r, tag="xr")
            for s in range(RING):
                nc.sync.dma_start(xr[32:33, s, :], bv_t.ap()[None, :])

            # zero-padded stationary arena: per k, slots [0..6], hT_k at
            # slot 3; stationary for (k, q) = arena[:, k, 3-q:7-q, :]
            arena = wpool.tile([128, 4, 7, B], f32r, tag="arena")
            nc.vector.memset(arena[:].bitcast(f32), 0.0)
            # persistent [c | tanh_g] tile; c lives in [:, 0:HH]
            tgc = wpool.tile([128, 2 * HH], f32, tag="tgc")
            nc.vector.memset(tgc[:], 0.0)

            def emit_idb(t, zp):
                s = t % RING
                for q in range(4):
                    nc.tensor.matmul(zp[:], idb_sb[:, q, :],
                                     xr[:, s, 512 * q:512 * (q + 1)],
                                     start=(q == 0), stop=False)

            # prefetch first xr slots
            for t0 in range(min(PREF, t_steps)):
                nc.sync.dma_start(xr[0:32, t0 % RING, :], xp_dram[t0, :, :])

            warm_ps = fpool.tile([128, 512], f32, tag="warm")
            nc.tensor.matmul(warm_ps[:], Wx_sb[:, 0, 0:128], Wx_sb[:, 2, 0:512],
                             start=True, stop=False, skip_group_check=True)

            zp = ppool.tile([128, 512], f32, tag="z")
            emit_idb(0, zp)

            for t in range(t_steps):
                tp = t + PREF
                if tp < t_steps:
                    nc.sync.dma_start(xr[0:32, tp % RING, :], xp_dram[tp, :, :])

                # contraction: for each k, 4 block-MMs (zero-padded cols)
                for k in range(4):
                    for q in range(4):
                        nc.tensor.matmul(zp[:],
                                         arena[:, k, 3 - q:7 - q, :],
                                         Wh_sb[:, k, 512 * q:512 * (q + 1)],
                                         start=False,
                                         stop=(k == 3 and q == 3))

                quanta = step_quanta.get(t, ())

                # idb round for t+1 fills more of the PE tail window
                if t + 1 < t_steps:
                    zp_next = ppool.tile([128, 512], f32, tag="z")
                    emit_idb(t + 1, zp_next)

                # --- tail (all ops on 128 partitions) ---
                sfi = tpool.tile([128, 2 * HH], f32, tag="sfi")
                nc.scalar.activation(sfi[:], zp[:, 0:2 * HH], AF.Sigmoid)
                nc.scalar.activation(tgc[:, HH:2 * HH], zp[:, 3 * HH:4 * HH],
                                     AF.Tanh)
                so = tpool.tile([128, HH], f32, tag="so")
                nc.scalar.activation(so[:], zp[:, 2 * HH:3 * HH], AF.Sigmoid)

                # filler matmuls on steps with no p1 work left: consume sfi
                # (so the scheduler keeps them in this step's tail window)
                # and accumulate into the anchored warm_ps group. They keep
                # the PE busy through the tail so the HAM clock gate never
                # re-throttles the next contraction block to 1.2 GHz.
                if not quanta and t + 1 < t_steps:
                    for _f in range(3):
                        nc.tensor.matmul(warm_ps[:, 0:2 * HH], ident[:],
                                         sfi[:], start=False, stop=False,
                                         skip_group_check=True)

                t12 = tpool.tile([128, 2 * HH], f32, tag="t12")
                nc.vector.tensor_mul(t12[:], sfi[:], tgc[:])
                nc.vector.tensor_add(tgc[:, 0:HH], t12[:, 0:HH], t12[:, HH:2 * HH])
                tcl = tpool.tile([128, HH], f32, tag="tc")
                nc.scalar.activation(tcl[:], tgc[:, 0:HH], AF.Tanh)
                hs = tpool.tile([128, HH], f32, tag="hs")
                nc.vector.tensor_mul(hs[:], so[:], tcl[:])

                # transpose h into stationary layout, write arena center
                # slots; per-k copies so each k's matmul group of the next
                # step can start as soon as its arena segment lands
                hT_ps = tppool.tile([128, 128], f32, tag="tp")
                nc.tensor.transpose(hT_ps[:], hs[:], ident[:])
                for k in range(4):
                    nc.vector.tensor_copy(arena[:, k, 3, :],
                                          hT_ps[:, 32 * k:32 * (k + 1)])

                nc.sync.dma_start(out_t.ap()[t].rearrange("(k p) b -> p k b", p=128),
                                  arena[:, :, 3, :])

                for (c, bq, nq) in quanta:
                    emit_p1_quarter(c, bq, nq)

                if t + 1 < t_steps:
                    zp = zp_next

            # close + anchor the filler accumulation group
            nc.tensor.matmul(warm_ps[:], Wx_sb[:, 0, 0:128], Wx_sb[:, 2, 0:512],
                             start=False, stop=True, skip_group_check=True)
            warm_sb = wpool.tile([1, 512], f32, tag="warmsb")
            nc.vector.tensor_copy(warm_sb[:], warm_ps[0:1, :])
            nc.sync.dma_start(xp_dram[0, 0:1, 0:512].bitcast(f32), warm_sb[:])

    nc.compile()
    return nc


def _get_program(t_steps=T, reps=1):
    key = (t_steps, reps)
    if key not in _PROG_CACHE:
        _PROG_CACHE[key] = _build_program(t_steps, reps)
    return _PROG_CACHE[key]


def _permute_gates(W, b):
    # reference gate order [i, f, o, g] (each H wide) -> kernel column order
    # per hidden chunk q (128 wide): [f_q | i_q | o_q | g_q]
    i_, f_, o_, g_ = (W[:, k * H:(k + 1) * H] for k in range(4))
    ib, fb, ob, gb = (b[k * H:(k + 1) * H] for k in range(4))
    HH = H // 4
    wcols, bcols = [], []
    for q in range(4):
        sl = slice(q * HH, (q + 1) * HH)
        wcols += [f_[:, sl], i_[:, sl], o_[:, sl], g_[:, sl]]
        bcols += [fb[sl], ib[sl], ob[sl], gb[sl]]
    return (np.ascontiguousarray(np.concatenate(wcols, axis=1)),
            np.ascontiguousarray(np.concatenate(bcols)))


LAST_EXEC_NS = None
LAST_TRACE = None


def _ensure_trace_hook():
    # The container's antenv stub lacks axon_hooks, so trn_boot's NTFF
    # profile hook registration silently degraded. Recreate it here.
    import types
    if "antenv.axon_hooks" in sys.modules:
        return
    mod = types.ModuleType("antenv.axon_hooks")
    _h = [None]
    mod.set_axon_ntff_profile_hook = lambda h: _h.__setitem__(0, h)
    mod.get_axon_ntff_profile_hook = lambda: _h[0]
    sys.modules["antenv.axon_hooks"] = mod
    import antenv
    antenv.axon_hooks = mod
    try:
        from trn_agent_boot.trn_boot import _ntff_profile_via_ctypes
        mod.set_axon_ntff_profile_hook(
            _ntff_profile_via_ctypes("/opt/axon/libaxon_pjrt.so"))
    except Exception as e:
        print(f"trace hook setup failed: {e}", file=sys.stderr)


def kernel(x, W_fw, b_fw, W_bw, b_bw, t_steps=None, trace=False):
    global LAST_EXEC_NS, LAST_TRACE
    if trace:
        _ensure_trace_hook()
    from concourse.bass_utils import run_bass_kernel_spmd

    x = np.asarray(x, dtype=np.float32)
    ts = t_steps or x.shape[1]
    nc = _get_program(ts)

    # block-diagonal idb: [:, q, 32q+b] = e_b rows + bias row of ones
    idb = np.zeros((33, 4, 128), np.float32)
    for q in range(4):
        idb[:32, q, 32 * q:32 * (q + 1)] = np.eye(32, dtype=np.float32)
        idb[32, q, 32 * q:32 * (q + 1)] = 1.0

    Wf, bf = _permute_gates(np.asarray(W_fw, np.float32), np.asarray(b_fw, np.float32))
    Wb, bb = _permute_gates(np.asarray(W_bw, np.float32), np.asarray(b_bw, np.float32))

    x_rev = np.ascontiguousarray(x[:, ::-1])

    core0 = {"x": x, "Wx": np.ascontiguousarray(Wf[:D]),
             "Wh": np.ascontiguousarray(Wf[D:]), "bv": bf, "idb": idb}
    core1 = {"x": x_rev, "Wx": np.ascontiguousarray(Wb[:D]),
             "Wh": np.ascontiguousarray(Wb[D:]), "bv": bb, "idb": idb}
    in_maps = [core0, core1] + [core0] * (N_CORES - 2)

    res = run_bass_kernel_spmd(nc, in_maps, list(range(N_CORES)), trace=trace)
    if res.exec_time_ns is not None:
        LAST_EXEC_NS = res.exec_time_ns
    if res.instructions_and_trace is not None:
        LAST_TRACE = res.instructions_and_trace[1]

    h_fw = res.results[0]["out_h"].transpose(2, 0, 1)          # [B, T, H]
    h_bw = res.results[1]["out_h"][::-1].transpose(2, 0, 1)
    return np.ascontiguousarray(
        np.concatenate([h_fw, h_bw], axis=-1).astype(np.float32))
